# revision 1
# baseline (speedup 1.0000x reference)
"""Trainium2 Bass kernel for nn_CausalSelfAttention_45200235823551.

Causal self-attention with low-rank key/value encoders:
  D=1024, H=16 heads, HD=64, F=32 freqs, KR=3, VR=192, B=2, S=2048.

Sharding: 8 cores, each core owns 2 heads x both batches (tensor parallel
over heads). Each core computes its heads' q/k/v, attention, and a partial
output projection (its heads' rows of Wproj); the host sums the 8 partial
outputs (row-parallel linear unshard).

Per-core layout ("T-major" = feature rows on partitions, sequence on free):
  - xT [1024, 2048] per batch, bf16
  - wcomb [1024, 640] = [Wq_my(128) | Wq_my_pairswap(128) | Wk(192) | Wv(192)]
  - one matmul pass produces qT, qswapT, zkT (key latents), xvT (value latents)
  - k decode: kT = A.T @ zkT with A[192,128] holding complex basis coefs,
    plus a pair-swapped variant for RoPE
  - RoPE applied in T-layout with precomputed cos/sin row tables
  - scoresT[sk,sq] = krotT.T @ qrotT per head (K=64, both heads row-packed)
  - exp on ScalarE (scale=1/8 folded in), causal mask by 0/1 multiply on
    diagonal blocks only
  - attn@v with v stationary: yT_aug[d+1, sq] accumulated over sk blocks;
    an all-ones column of v gives the softmax denominator for free
  - normalize via VectorE reciprocal + GpSimd partition_broadcast
  - partial projection: yout[sq, :] = ynT.T @ Wproj_rows
"""

import os
import sys

import numpy as np

sys.path.insert(0, "/opt/trn_rl_repo")

import ml_dtypes

D, H, HD = 1024, 16, 64
F, KR, VR = 32, 3, 192
B, S = 2, 2048
NCORE = 8
CH = 512          # sq chunk width
NCH = S // CH     # 4
BLK = 128         # sk block
NBLK = S // BLK   # 16
VW = 193          # v_sb per-block: [v_h0(64) | 1 | 1 | zeros(63) | v_h1(64)]
ROPE_BASE = 10000.0

_COMPILED = {}


def _build_bass():
    import concourse.bass as bass
    import concourse.tile as tile
    from concourse import mybir
    from contextlib import ExitStack

    BF = mybir.dt.bfloat16
    F32 = mybir.dt.float32
    AF = mybir.ActivationFunctionType

    nc = bass.Bass()
    xt = nc.dram_tensor("xt", [B, D, S], BF, kind="ExternalInput")
    wcomb = nc.dram_tensor("wcomb", [D, 640], BF, kind="ExternalInput")
    acoef = nc.dram_tensor("acoef", [192, 256], BF, kind="ExternalInput")
    vdeca = nc.dram_tensor("vdeca", [128, VW], BF, kind="ExternalInput")
    vdecb = nc.dram_tensor("vdecb", [128, VW], BF, kind="ExternalInput")
    wproj = nc.dram_tensor("wproj", [128, D], BF, kind="ExternalInput")
    cosT = nc.dram_tensor("cosT", [128, S], BF, kind="ExternalInput")
    sinT = nc.dram_tensor("sinT", [128, S], BF, kind="ExternalInput")
    maskc = nc.dram_tensor("maskc", [128, 4 * CH], BF, kind="ExternalInput")
    yout = nc.dram_tensor("yout", [B, S, D], F32, kind="ExternalOutput")
    dscr = nc.dram_tensor("dscr", [16, CH], F32)  # denom-recip bounce buffer

    with ExitStack() as ctx:
        tc = ctx.enter_context(tile.TileContext(nc))
        consts = ctx.enter_context(tc.tile_pool(name="consts", bufs=1))
        bigs = ctx.enter_context(tc.tile_pool(name="bigs", bufs=2))
        tmps = ctx.enter_context(tc.tile_pool(name="tmps", bufs=3))
        chunks = ctx.enter_context(tc.tile_pool(name="chunks", bufs=3))
        xpool = ctx.enter_context(tc.tile_pool(name="xpool", bufs=16))
        epool = ctx.enter_context(tc.tile_pool(name="epool", bufs=16))
        smalls = ctx.enter_context(tc.tile_pool(name="smalls", bufs=4))
        opool = ctx.enter_context(tc.tile_pool(name="opool", bufs=4))
        sppool = ctx.enter_context(tc.tile_pool(name="sppool", bufs=2, space="PSUM"))
        ypool = ctx.enter_context(tc.tile_pool(name="ypool", bufs=2, space="PSUM"))
        mmpool = ctx.enter_context(tc.tile_pool(name="mmpool", bufs=2, space="PSUM"))

        # ---- load constants ----
        wcomb_sb = []
        for kt in range(8):
            t = consts.tile([128, 640], BF, tag=f"wcomb{kt}")
            nc.sync.dma_start(out=t, in_=wcomb[kt * 128:(kt + 1) * 128, :])
            wcomb_sb.append(t)
        acoef0 = consts.tile([128, 256], BF, tag="acoef0")
        nc.sync.dma_start(out=acoef0, in_=acoef[0:128, :])
        acoef1 = consts.tile([64, 256], BF, tag="acoef1")
        nc.sync.dma_start(out=acoef1, in_=acoef[128:192, :])
        vdeca_sb = consts.tile([128, VW], BF, tag="vdeca")
        nc.sync.dma_start(out=vdeca_sb, in_=vdeca[:, :])
        vdecb_sb = consts.tile([128, VW], BF, tag="vdecb")
        nc.sync.dma_start(out=vdecb_sb, in_=vdecb[:, :])
        wproj_sb = consts.tile([128, D], BF, tag="wproj")
        nc.sync.dma_start(out=wproj_sb, in_=wproj[:, :])
        cos_sb = consts.tile([128, S], BF, tag="cos")
        nc.sync.dma_start(out=cos_sb, in_=cosT[:, :])
        sin_sb = consts.tile([128, S], BF, tag="sin")
        nc.sync.dma_start(out=sin_sb, in_=sinT[:, :])
        mask_sb = consts.tile([128, 4 * CH], BF, tag="mask")
        nc.sync.dma_start(out=mask_sb, in_=maskc[:, :])

        for b in range(B):
            # ---- phase A: projections, k decode, v decode ----
            q_sb = bigs.tile([128, S], BF, tag="q")
            qs_sb = bigs.tile([128, S], BF, tag="qs")
            k_sb = bigs.tile([128, S], BF, tag="k")
            ks_sb = bigs.tile([128, S], BF, tag="ks")
            v_sb = bigs.tile([128, NBLK * VW], BF, tag="v")
            yn_sb = bigs.tile([128, S], BF, tag="yn")

            for c in range(NCH):
                cs = slice(c * CH, (c + 1) * CH)
                xts = []
                for kt in range(8):
                    t = xpool.tile([128, CH], BF, tag="xt")
                    nc.gpsimd.dma_start(
                        out=t, in_=xt[b, kt * 128:(kt + 1) * 128, cs])
                    xts.append(t)
                # combined projection: 5 column tiles of wcomb
                zk0 = chunks.tile([128, CH], BF, tag="zk0")
                zk1 = chunks.tile([64, CH], BF, tag="zk1")
                xva = chunks.tile([128, CH], BF, tag="xva")  # rows 64:128 used
                xvb = chunks.tile([128, CH], BF, tag="xvb")
                for ct in range(5):
                    ps = mmpool.tile([128, CH], F32, tag="mm")
                    for kt in range(8):
                        nc.tensor.matmul(
                            ps, lhsT=wcomb_sb[kt][:, ct * 128:(ct + 1) * 128],
                            rhs=xts[kt], start=(kt == 0), stop=(kt == 7))
                    if ct == 0:
                        nc.vector.tensor_copy(q_sb[:, cs], ps)
                    elif ct == 1:
                        nc.vector.tensor_copy(qs_sb[:, cs], ps)
                    elif ct == 2:
                        nc.vector.tensor_copy(zk0, ps)
                    elif ct == 3:
                        nc.vector.tensor_copy(zk1, ps[0:64, :])
                        nc.vector.tensor_copy(xva[64:128, :], ps[64:128, :])
                    else:
                        nc.vector.tensor_copy(xvb, ps)
                # k decode (and pair-swapped variant)
                psk = mmpool.tile([128, CH], F32, tag="mm")
                nc.tensor.matmul(psk, lhsT=acoef0[:, 0:128], rhs=zk0,
                                 start=True, stop=False)
                nc.tensor.matmul(psk, lhsT=acoef1[:, 0:128], rhs=zk1,
                                 start=False, stop=True)
                nc.vector.tensor_copy(k_sb[:, cs], psk)
                psks = mmpool.tile([128, CH], F32, tag="mm")
                nc.tensor.matmul(psks, lhsT=acoef0[:, 128:256], rhs=zk0,
                                 start=True, stop=False)
                nc.tensor.matmul(psks, lhsT=acoef1[:, 128:256], rhs=zk1,
                                 start=False, stop=True)
                nc.vector.tensor_copy(ks_sb[:, cs], psks)
                # v decode: per 128-seq block, both heads side by side
                for j in range(4):
                    sb = 4 * c + j
                    js = slice(j * BLK, (j + 1) * BLK)
                    psv = mmpool.tile([128, VW], F32, tag="mm")
                    nc.tensor.matmul(psv, lhsT=xva[64:128, js],
                                     rhs=vdeca_sb[64:128, :],
                                     start=True, stop=False)
                    nc.tensor.matmul(psv, lhsT=xvb[:, js], rhs=vdecb_sb,
                                     start=False, stop=True)
                    vs = slice(sb * VW, (sb + 1) * VW)
                    nc.vector.tensor_copy(v_sb[:, vs], psv)
                    nc.vector.memset(v_sb[:, sb * VW + 64:sb * VW + 66], 1.0)

            # ---- RoPE ----
            t1 = tmps.tile([128, S], BF, tag="tmp")
            nc.vector.tensor_mul(t1, q_sb, cos_sb)
            nc.vector.tensor_mul(qs_sb, qs_sb, sin_sb)
            nc.vector.tensor_add(q_sb, t1, qs_sb)
            t2 = tmps.tile([128, S], BF, tag="tmp")
            nc.vector.tensor_mul(t2, k_sb, cos_sb)
            nc.vector.tensor_mul(ks_sb, ks_sb, sin_sb)
            nc.vector.tensor_add(k_sb, t2, ks_sb)

            # ---- phase B: attention + partial projection ----
            for c in range(NCH):
                cs = slice(c * CH, (c + 1) * CH)
                nblk = 4 * (c + 1)
                egroups = {0: [], 1: []}
                for g0 in range(0, nblk, 2):
                    gw = min(2, nblk - g0)
                    for h in (0, 1):
                        hp = slice(h * 64, (h + 1) * 64)
                        sp = sppool.tile([128, 2 * CH], F32, tag="score")
                        for i in range(gw):
                            blk = g0 + i
                            nc.tensor.matmul(
                                sp[:, i * CH:(i + 1) * CH],
                                lhsT=k_sb[hp, blk * BLK:(blk + 1) * BLK],
                                rhs=q_sb[hp, cs], start=True, stop=True)
                        et = epool.tile([128, 2 * CH], BF, tag="exp")
                        nc.scalar.activation(et[:, 0:gw * CH], sp[:, 0:gw * CH],
                                             AF.Exp, scale=0.125)
                        for i in range(gw):
                            blk = g0 + i
                            if blk >= 4 * c:  # diagonal block: causal mask
                                m = blk - 4 * c
                                nc.vector.tensor_mul(
                                    et[:, i * CH:(i + 1) * CH],
                                    et[:, i * CH:(i + 1) * CH],
                                    mask_sb[:, m * CH:(m + 1) * CH])
                        egroups[h].append(et)
                for h in (0, 1):
                    yp = ypool.tile([128, CH], F32, tag="y")
                    if h == 0:
                        oslc, dslc, rslc = slice(0, 65), slice(0, 64), slice(64, 65)
                        vcol = 0, 65
                    else:
                        oslc, dslc, rslc = slice(0, 128), slice(64, 128), slice(0, 1)
                        vcol = 65, 193
                    for blk in range(nblk):
                        et = egroups[h][blk // 2]
                        off = (blk % 2) * CH
                        nc.tensor.matmul(
                            yp[oslc, :],
                            lhsT=v_sb[:, blk * VW + vcol[0]:blk * VW + vcol[1]],
                            rhs=et[:, off:off + CH],
                            start=(blk == 0), stop=(blk == nblk - 1))
                    rc = smalls.tile([128, CH], F32, tag="recip")
                    nc.vector.reciprocal(rc[rslc, :], yp[rslc, :])
                    di = b * 8 + c * 2 + h
                    nc.sync.dma_start(out=dscr[di:di + 1, :], in_=rc[rslc, :])
                    bc = smalls.tile([128, CH], F32, tag="bc")
                    nc.gpsimd.dma_start(
                        out=bc[dslc, :],
                        in_=dscr[di:di + 1, :].to_broadcast([64, CH]))
                    nc.vector.tensor_mul(yn_sb[dslc, cs], yp[dslc, :], bc[dslc, :])
                # partial output projection for this chunk
                for j in range(4):
                    sb = 4 * c + j
                    for n in range(2):
                        pp = mmpool.tile([128, CH], F32, tag="mm")
                        nc.tensor.matmul(
                            pp, lhsT=yn_sb[:, sb * BLK:(sb + 1) * BLK],
                            rhs=wproj_sb[:, n * CH:(n + 1) * CH],
                            start=True, stop=True)
                        ot = opool.tile([128, CH], F32, tag="out")
                        nc.vector.tensor_copy(ot, pp)
                        nc.sync.dma_start(
                            out=yout[b, sb * BLK:(sb + 1) * BLK,
                                     n * CH:(n + 1) * CH],
                            in_=ot)
    _split_dma_waits(nc, mybir)
    return nc


def _split_dma_waits(nc, mybir):
    """This container's walrus rejects instructions whose 64B encoding lacks
    room for their sem waits ("Too many sync wait commands"): DMAs and NoOps
    hold 1 wait, matmuls 2. Hoist excess waits onto a chain of single-wait
    NoOps in the same engine stream directly before the instruction — the
    sequencer blocks on each, which is semantically identical."""
    cap = {}
    f = nc.m.functions[0]
    blocks = f.body if hasattr(f, "body") else f.blocks
    n = 0
    for blk in blocks:
        insts = list(blk.instructions)
        out = []
        changed = False
        for inst in insts:
            si = inst.sync_info
            tn = type(inst).__name__
            limit = cap.get(tn, 1)
            if si is not None and si.on_wait and len(si.on_wait) > limit:
                waits = list(si.on_wait)
                keep = waits[-limit:]
                for w in waits[:-limit]:
                    nop = mybir.InstNoOp(name=f"I-dmaw-{n}")
                    n += 1
                    nop.engine = inst.engine
                    nop.sync_info = mybir.SyncInfo(on_wait=[w], on_update=[])
                    nc.register_instruction(nop)
                    out.append(nop)
                inst.sync_info = mybir.SyncInfo(
                    on_wait=keep, on_update=list(si.on_update or []))
                changed = True
            out.append(inst)
        if changed:
            if hasattr(blk, "set_instructions"):
                blk.set_instructions(out)
            else:
                try:
                    blk.instructions = out
                except Exception:
                    blk.instructions[:] = out
    return nc


def _host_inputs(x, Wq, Wk, Wv, key_decoder, value_decoder, Wproj):
    bf16 = ml_dtypes.bfloat16
    x = np.asarray(x, np.float32)
    Wq = np.asarray(Wq, np.float32)
    Wk = np.asarray(Wk, np.float32)
    Wv = np.asarray(Wv, np.float32)
    key_decoder = np.asarray(key_decoder, np.float32)
    value_decoder = np.asarray(value_decoder, np.float32)
    Wproj = np.asarray(Wproj, np.float32)

    xt = np.ascontiguousarray(x.transpose(0, 2, 1)).astype(bf16)  # [B, D, S]

    half = HD // 2
    freq = 1.0 / (ROPE_BASE ** (np.arange(half, dtype=np.float32) / half))
    th = np.outer(np.arange(S, dtype=np.float32), freq)  # [S, 32]
    cos, sin = np.cos(th), np.sin(th)
    rows = np.arange(128)
    fidx = (rows % 64) // 2
    cosT = cos[:, fidx].T.astype(bf16)                       # [128, S]
    sgn = np.where(rows % 2 == 0, -1.0, 1.0)[:, None]
    sinT = (sin[:, fidx].T * sgn).astype(bf16)

    maskc = np.zeros((128, 4 * CH), np.float32)
    p = np.arange(128)[:, None]
    j = np.arange(CH)[None, :]
    for m in range(4):
        maskc[:, m * CH:(m + 1) * CH] = (p <= j - 128 * m)
    maskc = maskc.astype(bf16)

    Wq4 = Wq.reshape(D, H, HD)
    br, bi = key_decoder[..., 0], key_decoder[..., 1]  # [F, H, KR]

    in_maps = []
    for core in range(NCORE):
        h0, h1 = 2 * core, 2 * core + 1
        wq_my = Wq4[:, [h0, h1], :].reshape(D, 128)
        wq_sw = np.ascontiguousarray(
            Wq4[:, [h0, h1], :].reshape(D, 2, 32, 2)[..., ::-1]).reshape(D, 128)
        wcomb = np.concatenate([wq_my, wq_sw, Wk, Wv], axis=1).astype(bf16)

        A = np.zeros((192, 128), np.float32)
        for hl, h in enumerate((h0, h1)):
            for f in range(F):
                for r in range(KR):
                    A[f * 6 + r * 2 + 0, hl * 64 + 2 * f] = br[f, h, r]
                    A[f * 6 + r * 2 + 1, hl * 64 + 2 * f] = -bi[f, h, r]
                    A[f * 6 + r * 2 + 0, hl * 64 + 2 * f + 1] = bi[f, h, r]
                    A[f * 6 + r * 2 + 1, hl * 64 + 2 * f + 1] = br[f, h, r]
        Asw = np.ascontiguousarray(
            A.reshape(192, 2, 32, 2)[..., ::-1]).reshape(192, 128)
        acoef = np.concatenate([A, Asw], axis=1).astype(bf16)

        vdeca = np.zeros((128, VW), np.float32)
        vdecb = np.zeros((128, VW), np.float32)
        vdeca[64:128, 0:64] = value_decoder[h0][0:64, :]
        vdeca[64:128, 129:193] = value_decoder[h1][0:64, :]
        vdecb[:, 0:64] = value_decoder[h0][64:192, :]
        vdecb[:, 129:193] = value_decoder[h1][64:192, :]

        wproj_my = np.concatenate(
            [Wproj[h0 * 64:(h0 + 1) * 64, :], Wproj[h1 * 64:(h1 + 1) * 64, :]],
            axis=0).astype(bf16)

        in_maps.append({
            "xt": xt, "wcomb": wcomb, "acoef": acoef,
            "vdeca": vdeca.astype(bf16), "vdecb": vdecb.astype(bf16),
            "wproj": wproj_my, "cosT": cosT, "sinT": sinT, "maskc": maskc,
        })
    return in_maps


def kernel(x, Wq, Wk, Wv, key_decoder, value_decoder, Wproj):
    from concourse.bass_utils import run_bass_kernel_spmd

    if "nc" not in _COMPILED:
        _COMPILED["nc"] = _build_bass()
    nc = _COMPILED["nc"]

    in_maps = _host_inputs(x, Wq, Wk, Wv, key_decoder, value_decoder, Wproj)
    import time as _time
    t0 = _time.time()
    res = run_bass_kernel_spmd(nc, in_maps, list(range(NCORE)))
    _COMPILED["exec_wall_ns"] = (_time.time() - t0) * 1e9
    _COMPILED["last_result"] = res
    out = np.zeros((B, S, D), np.float64)
    for r in res.results:
        out += r["yout"].astype(np.float64)
    return out.astype(np.float32)

